# revision 22
# baseline (speedup 1.0000x reference)
"""Trainium2 Bass kernel for multi-head attention with relative position bias.

Problem: x[8,1024,768] -> qkv proj -> 12-head attention (+rel_pos_bias,
softmax) -> out proj.  Sharding: pure batch-parallel, 1 batch per core on
8 cores, zero collectives; weights/bias replicated.

v3 (on top of v2's 128x128-tile-mode / padded-K scores / [v|ones] attn@v):
  - es/eb/p merged per t: ONE [128,2048] exp-bias tile per (nh,hp,t)
    (host interleaves A|B heads), ONE DVE multiply per t instead of two.
  - v laid out at stride 66 (dims 0:64, ones col 64, pad col 65) so the
    v' evacuation is 4B-aligned; one [128,768] psum chain per m-chunk.
  - sums land in recip-layout tiles r_acc[16c:16c+16, 64] directly from
    stg row 64 (no sums_row bounce hop).
  - DMA order: x/wv + qkw pairs 0-2 upfront; pairs 3-5 and pw dripped
    during attention so the first expB tiles arrive ~10us earlier.
  - nh1 chunks processed [1,2,3,4,5,0]; chunks 1-5 normalized inside
    pos5's t-slots; 4 of 6 final proj groups pre-opened as cc1-5
    partials so the tail is ~6 matmuls + short recip chain.

Per-core layout (host pre-transposes, free):
  - xT      [128, 6(cc) x 1024]     bf16  x[b].T chunked on contraction dim
  - qkwT    [128, 12(oc)x6(cc)x128] bf16  (q,k weights).T, q pre-scaled
  - wvT     [128, 6(cc) x 768]      bf16  v weights .T
  - expBT   [6(hp), 128, 2(nh)x4(t)x2048]  bf16  exp(bias) interleaved A|B
  - pwT     [128, 6(oc)x6(cc)x128]  bf16  proj_w.T
  - pb      [128, 6]                fp32  proj_b per o-chunk
  out zT    [128, 6(oc) x 1024]     bf16  z.T
"""

import numpy as np
from contextlib import ExitStack

import concourse.bass as bass
import concourse.mybir as mybir
from concourse import bacc
from concourse import tile

F32 = mybir.dt.float32
BF16 = mybir.dt.bfloat16

P = 128
N = 1024          # sequence
C = 768           # dim
H = 12            # heads
DH = 64           # head dim
NCC = 6           # contraction chunks (768/128)
NOC_QK = 12       # q,k output chunks (1536/128)
NM = 8            # seq chunks (1024/128)
VS = 66           # v group stride: [v 0:64 | ones 64 | pad 65]
SCALE = DH ** -0.5
EXPF = mybir.ActivationFunctionType.Exp


def build_nc():
    nc = bacc.Bacc(None, target_bir_lowering=False, debug=False)
    xT = nc.declare_dram_parameter("xT", [P, NCC * N], BF16, isOutput=False)
    qkwT = nc.declare_dram_parameter("qkwT", [P, NOC_QK * NCC * P], BF16, isOutput=False)
    wvT = nc.declare_dram_parameter("wvT", [P, NCC * C], BF16, isOutput=False)
    expBT = nc.declare_dram_parameter("expBT", [NCC, P, 2 * 4 * 2048], BF16, isOutput=False)
    pwT = nc.declare_dram_parameter("pwT", [P, NCC * NCC * P], BF16, isOutput=False)
    pb = nc.declare_dram_parameter("pb", [P, NCC], F32, isOutput=False)
    zT = nc.declare_dram_parameter("zT", [P, NCC * N], BF16, isOutput=True)

    with tile.TileContext(nc) as tc, ExitStack() as ctx:
        big = ctx.enter_context(tc.tile_pool(name="big", bufs=1))
        ebpool = ctx.enter_context(tc.tile_pool(name="eb", bufs=6))
        espool = ctx.enter_context(tc.tile_pool(name="es", bufs=3))
        ppool = ctx.enter_context(tc.tile_pool(name="p", bufs=3))
        stgpool = ctx.enter_context(tc.tile_pool(name="stg", bufs=3))
        rpool = ctx.enter_context(tc.tile_pool(name="r", bufs=1))
        drampool = ctx.enter_context(tc.tile_pool(name="dram", bufs=1, space="DRAM"))
        ps_pool = ctx.enter_context(tc.tile_pool(name="psum", bufs=2, space="PSUM"))
        pu_pool = ctx.enter_context(tc.tile_pool(name="psum_u", bufs=1, space="PSUM"))

        # persistent SBUF tensors
        q_sb = big.tile([P, NCC * N], BF16, tag="q")            # 12KB/part
        kpad_sb = big.tile([P, NOC_QK * N], BF16, tag="kpad")   # 24KB
        v_sb = big.tile([P, NM * H * VS], BF16, tag="vz")       # 12.4KB
        x_sb = big.tile([P, NCC * N], BF16, tag="xr")           # 12KB
        y_sb = big.tile([P, NCC * N], BF16, tag="y")            # 12KB
        qkw_sb = big.tile([P, NOC_QK * NCC * P], BF16, tag="qkw")  # 18KB
        wv_sb = big.tile([P, NCC * C], BF16, tag="wv")          # (shared w/ pw: 18KB)
        pb_sb = big.tile([P, NCC], F32, tag="pbt")

        # ---------------- phase 1 lead-in ---------------------------------
        # DMA priority: x + wv as single large transfers (v'/qk gate on
        # them), then qkw pairs 0-2, pb.  qkw pairs 3-5 and pw are emitted
        # later, inside the attention stream, so the first expB tiles
        # aren't stuck behind 7MB of queue.
        for cc in range(NCC):
            nc.sync.dma_start(x_sb[:, cc * N:(cc + 1) * N], xT[:, cc * N:(cc + 1) * N])
            nc.sync.dma_start(wv_sb[:, cc * C:(cc + 1) * C], wvT[:, cc * C:(cc + 1) * C])

        def load_qkw_pair(hp):
            for oc in (NCC + hp, hp):
                nc.sync.dma_start(qkw_sb[:, oc * NCC * P:(oc + 1) * NCC * P],
                                  qkwT[:, oc * NCC * P:(oc + 1) * NCC * P])

        for hp_ in range(3):
            load_qkw_pair(hp_)
        nc.sync.dma_start(pb_sb[:], pb[:])

        # zero the pad halves of kpad (k evacs later fill the live halves);
        # gpsimd is otherwise idle, keep this off the DVE critical path
        nc.gpsimd.memset(kpad_sb[:], 0.0)

        # ones column at 64 (so attention row-sums land on PSUM partition 64
        # and the v' evacuation stays 4B-aligned); col 65 is dead padding
        v_view = v_sb[:].rearrange("p (k d) -> p k d", d=VS)
        nc.vector.memset(v_view[:, :, 64:65], 1.0)
        nc.gpsimd.memset(v_view[:, :, 65:66], 0.0)

        # selector for the tail normalize: bcast row 0 via K=65 matmul
        # (tile mode stays 128x128); r12pad holds the reciprocal row with
        # zeroed pad rows (so the K=65 contraction sees no garbage)
        sel_sb = big.tile([65, P], BF16, tag="sel")
        nc.gpsimd.memset(sel_sb[:], 0.0)
        nc.vector.memset(sel_sb[0:1, :], 1.0)
        r12pad = big.tile([65, N], BF16, tag="r12pad")
        nc.gpsimd.memset(r12pad[:], 0.0)

        # per-nh recip-layout sums accumulators: chunk c's sums live at
        # rows [16c, 16c+16) x 64 (written straight from stg row 64).
        # memset so the full-tile reciprocal below never sees raw SBUF.
        r_acc = [big.tile([96, 64], BF16, tag=f"racc{i}", name=f"r_acc{i}")
                 for i in range(2)]
        nc.gpsimd.memset(r_acc[0][:], 1.0)
        nc.gpsimd.memset(r_acc[1][:], 1.0)

        # v projection (lead-in; attention needs all of v')
        for j in range(NM):
            psv = ps_pool.tile([P, C], F32, tag="ps", bufs=2, name=f"psv_{j}")
            for cc in range(NCC):
                lhs = x_sb[:, cc * N + j * P: cc * N + (j + 1) * P]
                nc.tensor.matmul(psv[:, 0:512], lhs, wv_sb[:, cc * C: cc * C + 512],
                                 start=(cc == 0), stop=(cc == NCC - 1))
                nc.tensor.matmul(psv[:, 512:768], lhs, wv_sb[:, cc * C + 512: (cc + 1) * C],
                                 start=(cc == 0), stop=(cc == NCC - 1))
            nc.vector.tensor_copy(
                v_view[:, j * H:(j + 1) * H, 0:64],
                psv[:].rearrange("p (h d) -> p h d", d=64))

        def qk_group(oc, qnh):
            ps = ps_pool.tile([P, 512], F32, tag="psz", bufs=2,
                              name=f"psqk_{oc}_{qnh}")
            for cc in range(NCC):
                nc.tensor.matmul(
                    ps[:],
                    qkw_sb[:, (oc * NCC + cc) * P:(oc * NCC + cc + 1) * P],
                    x_sb[:, cc * N + qnh * 512: cc * N + qnh * 512 + 512],
                    start=(cc == 0), stop=(cc == NCC - 1),
                )
            if oc < NCC:  # q chunk: both heads stacked, single copy
                nc.vector.tensor_copy(
                    q_sb[:, oc * N + qnh * 512: oc * N + qnh * 512 + 512], ps[:])
            else:         # k chunk: split into zero-padded A/B regions
                hp = oc - NCC
                nc.vector.tensor_copy(
                    kpad_sb[0:64, hp * N + qnh * 512: hp * N + qnh * 512 + 512],
                    ps[0:64, :])
                nc.vector.tensor_copy(
                    kpad_sb[64:128, oc * N + qnh * 512: oc * N + qnh * 512 + 512],
                    ps[64:128, :])

        def proj_accum(psz, pnh, oc, ccs, first):
            for cc in ccs:
                nc.tensor.matmul(
                    psz[:],
                    pw_sb[:, (oc * NCC + cc) * P:(oc * NCC + cc + 1) * P],
                    y_sb[:, cc * N + pnh * 512: cc * N + pnh * 512 + 512],
                    start=first, stop=False,
                )
                first = False

        def proj_partial(pnh, oc, tag="psz", last_cc=0, pool=None, bufs=2):
            """Open a proj accumulation over all cc except last_cc."""
            psz = (pool or ps_pool).tile([P, 512], F32, tag=tag, bufs=bufs,
                                         name=f"psz_{pnh}_{oc}")
            proj_accum(psz, pnh, oc,
                       [cc for cc in range(NCC) if cc != last_cc], True)
            return psz

        def proj_final(psz, pnh, oc, last_cc=0):
            nc.tensor.matmul(
                psz[:],
                pw_sb[:, (oc * NCC + last_cc) * P:(oc * NCC + last_cc + 1) * P],
                y_sb[:, last_cc * N + pnh * 512: last_cc * N + pnh * 512 + 512],
                start=False, stop=True,
            )
            nc.vector.tensor_scalar_add(
                zT_sb[:, oc * N + pnh * 512: oc * N + pnh * 512 + 512],
                psz[:], pb_sb[:, oc:oc + 1])
            nc.sync.dma_start(
                zT[:, oc * N + pnh * 512: oc * N + pnh * 512 + 512],
                zT_sb[:, oc * N + pnh * 512: oc * N + pnh * 512 + 512])

        def proj_group(pnh, oc):
            psz = proj_partial(pnh, oc, last_cc=0)
            proj_final(psz, pnh, oc, last_cc=0)

        proj_open = {}

        def drip(items):
            for kind in items:
                if kind[0] == "projA":    # open psz, accumulate cc 1-3
                    oc = kind[1]
                    psz = ps_pool.tile([P, 512], F32, tag="psz", bufs=2,
                                       name=f"psz_0_{oc}")
                    proj_accum(psz, 0, oc, [1, 2, 3], True)
                    proj_open[oc] = psz
                elif kind[0] == "projB":  # cc 4,5 then close with cc0
                    oc = kind[1]
                    proj_accum(proj_open[oc], 0, oc, [4, 5], False)
                    proj_final(proj_open.pop(oc), 0, oc, last_cc=0)
                else:
                    qk_group(kind[1], kind[2])

        from collections import deque
        # pair 0's qnh0 k/q in the lead-in; the rest dripped into the
        # attention stream just ahead of need (q qnh1 halves are only
        # read in nh1, so they fill nh0's late, otherwise-idle slots)
        qk_group(NCC, 0)
        qk_group(0, 0)
        K_ = NCC  # k chunk oc offset
        fill = deque([
            [("qk", K_ + 0, 1), ("qk", K_ + 1, 0)],   # s0
            [("qk", K_ + 1, 1)], [("qk", 1, 0)],      # s1-2
            [("qk", K_ + 2, 0)], [("qk", K_ + 2, 1)], [("qk", 2, 0)],
            [("qk", K_ + 3, 0)], [("qk", K_ + 3, 1)], [("qk", 3, 0)],
            [("qk", K_ + 4, 0)], [("qk", K_ + 4, 1)], [("qk", 4, 0)],
            [("qk", K_ + 5, 0)], [("qk", K_ + 5, 1)], [("qk", 5, 0)],
            [("qk", 1, 1)], [("qk", 2, 1)], [("qk", 3, 1)],
            [("qk", 4, 1)], [("qk", 5, 1)], [("qk", 0, 1)],            # s15-20
        ])

        # ---------------- phase 2: attention, n-half outer ----------------
        pw_sb = big.tile([P, NCC * NCC * P], BF16, tag="wv")  # wv dead after phase 1
        zT_sb = big.tile([P, NCC * N], BF16, tag="zt")

        R_sb = big.tile([P, NCC * N], BF16, tag="xr")   # reuse x slot

        # Deferred weight DMAs, dripped at chunk starts so they don't
        # delay the first expB tiles in the DMA queues.
        def load_pw_half(half):
            hw = NCC * NCC * P // 2
            nc.sync.dma_start(pw_sb[:, half * hw:(half + 1) * hw],
                              pwT[:, half * hw:(half + 1) * hw])

        chunk_dmas = {
            (0, 0): lambda: load_qkw_pair(3),
            (0, 1): lambda: load_qkw_pair(4),
            (0, 2): lambda: (load_qkw_pair(5), load_pw_half(0)),
            (0, 3): lambda: load_pw_half(1),
        }

        # Batched normalize for chunk range [c0, c1) of n-half nh, staged
        # across t-slots so each engine op is emitted only after its deps
        # retired (engine queues are FIFO; an op with pending deps blocks
        # its queue).  Stage A: reciprocal of r_acc rows + DRAM bounce out.
        # Stage B: broadcast back into R_sb.  Stage C: y *= R.
        def norm_stageA(nh, c0, c1):
            # DVE partition base must be 32-aligned: reciprocal the whole
            # 96-row tile (rows of absent chunks hold 1.0 from the memset),
            # then bounce only the live rows out through DRAM.
            nch = c1 - c0
            r12 = rpool.tile([96, 64], BF16, tag="r12", bufs=2,
                             name=f"r12_{nh}_{c0}")
            with nc.allow_low_precision(reason="bf16 recip of softmax sums"):
                nc.vector.reciprocal(r12[:], r_acc[nh][:])
            r_d = drampool.tile([16 * nch, 64], BF16, bufs=2,
                                name=f"r_d_{nh}_{c0}")
            # norm-chain DMAs ride the idle gpsimd queue: the sync queue
            # carries the expB stream and has ~10us head-of-line latency
            nc.gpsimd.dma_start(r_d[:], r12[16 * c0:16 * c1, :])
            return r_d

        def norm_stageB(nh, r_d, c0, c1):
            nch = c1 - c0
            r_pairs = (r_d[:].rearrange("l n -> (l n)")
                       .rearrange("(c n) -> c n", n=N))
            for par in range(2):  # even heads -> parts 0:64, odd -> 64:128
                nc.gpsimd.dma_start(
                    R_sb[64 * par: 64 * par + 64, :]
                    .rearrange("p (c n) -> p c n", n=N)
                    [:, c0:c1, nh * 512:nh * 512 + 512],
                    r_pairs[:, par * 512:par * 512 + 512][None, :, :]
                    .broadcast_to([64, nch, 512]),
                )

        def norm_stageC(nh, c0, c1):
            # per-chunk (512-wide) so no single DVE op piles onto the
            # queue ahead of the attention multiplies
            yh = (y_sb[:].rearrange("p (c n) -> p c n", n=N)
                  [:, c0:c1, nh * 512:(nh + 1) * 512])
            Rh = (R_sb[:].rearrange("p (c n) -> p c n", n=N)
                  [:, c0:c1, nh * 512:(nh + 1) * 512])
            nc.vector.tensor_mul(yh, yh, Rh)

        # staged-normalize / tail-prep slots: {(nh, pos, t): callable}
        norm_slots = {}
        post_slots = {}
        tail_psz = {}
        st = {}

        # nh0 chunks in order 0..5; nh1 in order 1..5,0 so the straggler
        # chunk is 0 and chunks [1,6) batch-normalize contiguously.
        for nh in range(2):
            order = list(range(NCC)) if nh == 0 else [1, 2, 3, 4, 5, 0]
            for pos, hp in enumerate(order):
                fn = chunk_dmas.pop((nh, pos), None)
                if fn is not None:
                    fn()
                hA, hB = 2 * hp, 2 * hp + 1
                u = pu_pool.tile([65, N], F32, tag="u", name=f"u_{nh}_{hp}")
                rhs_q = q_sb[:, hp * N + nh * 512: hp * N + nh * 512 + 512]
                for t in range(4):
                    for fn in norm_slots.pop((nh, pos, t), []):
                        fn()
                    ebAB = ebpool.tile([P, 2048], BF16, tag="eb")
                    nc.sync.dma_start(
                        ebAB[:], expBT[hp, :, nh * 8192 + t * 2048:
                                       nh * 8192 + (t + 1) * 2048])
                    psA = ps_pool.tile([P, N], F32, tag="ps", bufs=2,
                                       name=f"psA_{nh}_{hp}_{t}")
                    psB = ps_pool.tile([P, N], F32, tag="ps", bufs=2,
                                       name=f"psB_{nh}_{hp}_{t}")
                    for s in range(2):
                        j = 2 * t + s
                        kA = kpad_sb[:, hp * N + j * P: hp * N + (j + 1) * P]
                        kB = kpad_sb[:, (NCC + hp) * N + j * P: (NCC + hp) * N + (j + 1) * P]
                        nc.tensor.matmul(psA[:, s * 512:(s + 1) * 512], kA, rhs_q,
                                         start=True, stop=True)
                        nc.tensor.matmul(psB[:, s * 512:(s + 1) * 512], kB, rhs_q,
                                         start=True, stop=True)
                    # drip deferred matmul groups into the stream
                    if fill:
                        drip(fill.popleft())
                    fn = post_slots.pop((nh, pos, t), None)
                    if fn is not None:
                        fn()
                    esAB = espool.tile([P, 2048], BF16, tag="esAB")
                    nc.scalar.activation(esAB[:, 0:1024], psA[:], EXPF)
                    nc.scalar.activation(esAB[:, 1024:2048], psB[:], EXPF)
                    pAB = ppool.tile([P, 2048], BF16, tag="pAB")
                    nc.vector.tensor_mul(pAB[:], esAB[:], ebAB[:])
                    for s in range(2):
                        j = 2 * t + s
                        vA = v_sb[:, (j * H + hA) * VS: (j * H + hA) * VS + 65]
                        vB = v_sb[:, (j * H + hB) * VS: (j * H + hB) * VS + 65]
                        nc.tensor.matmul(u[:, 0:512], vA, pAB[:, s * 512:(s + 1) * 512],
                                         start=(j == 0), stop=(j == NM - 1),
                                         skip_group_check=True)
                        nc.tensor.matmul(u[:, 512:1024], vB,
                                         pAB[:, 1024 + s * 512: 1024 + (s + 1) * 512],
                                         start=(j == 0), stop=(j == NM - 1),
                                         skip_group_check=True)
                # evacuate u via one copy; DMAs scatter y halves + sums
                # (u rows 0:64 = head dims, row 64 = sums)
                ys = hp * N + nh * 512
                stg = stgpool.tile([65, N], BF16, tag="stg")
                nc.vector.tensor_copy(stg[:], u[:])
                nc.sync.dma_start(y_sb[0:64, ys: ys + 512], stg[0:64, 0:512])
                nc.sync.dma_start(y_sb[64:128, ys: ys + 512], stg[0:64, 512:1024])
                nc.gpsimd.dma_start(
                    r_acc[nh][16 * hp:16 * hp + 16, :],
                    stg[64:65, :])
                if nh == 0 and pos == NCC - 2:
                    # nh0 chunks 0-4 normalize staged into pos5's t-slots
                    norm_slots[(0, NCC - 1, 1)] = [
                        lambda: st.update(rd0=norm_stageA(0, 0, NCC - 1))]
                    norm_slots[(0, NCC - 1, 3)] = [
                        lambda: norm_stageB(0, st["rd0"], 0, NCC - 1)]
            if nh == 0:
                # nh0 normalize: batched recip/bounce done in pos5's slots;
                # the per-chunk y-multiplies spread one per slot across
                # nh1's first iterations (needed by slot 30's proj drips).
                # nh0 chunk 5's own bounce rides nh1-pos0's slots.
                norm_slots[(1, 0, 0)] = [lambda: norm_stageC(0, 0, 1)]
                norm_slots[(1, 0, 1)] = [
                    lambda: st.update(rd05=norm_stageA(0, NCC - 1, NCC)),
                    lambda: norm_stageC(0, 1, 2)]
                norm_slots[(1, 0, 2)] = [lambda: norm_stageC(0, 2, 3)]
                norm_slots[(1, 0, 3)] = [
                    lambda: norm_stageB(0, st["rd05"], NCC - 1, NCC),
                    lambda: norm_stageC(0, 3, 4)]
                norm_slots[(1, 1, 0)] = [lambda: norm_stageC(0, 4, 5)]
                norm_slots[(1, 1, 1)] = [lambda: norm_stageC(0, NCC - 1, NCC)]
                # nh1 normalize pipelined in two waves (chunks 1-4 during
                # pos4, chunk 5 during pos5) so the proj(1,*) cc1-5
                # partials can pre-run before straggler chunk 0 finishes
                norm_slots[(1, 4, 0)] = [
                    lambda: st.update(rd14=norm_stageA(1, 1, 5))]
                norm_slots[(1, 4, 1)] = [
                    lambda: norm_stageB(1, st["rd14"], 1, 5)]
                norm_slots[(1, 4, 2)] = [lambda: norm_stageC(1, 1, 2)]
                norm_slots[(1, 4, 3)] = [lambda: norm_stageC(1, 2, 3)]
                norm_slots[(1, 5, 0)] = [
                    lambda: norm_stageC(1, 3, 4),
                    lambda: st.update(rd15=norm_stageA(1, 5, 6))]
                norm_slots[(1, 5, 1)] = [
                    lambda: norm_stageC(1, 4, 5),
                    lambda: norm_stageB(1, st["rd15"], 5, 6)]
                norm_slots[(1, 5, 2)] = [lambda: norm_stageC(1, 5, 6)]
                norm_slots[(1, 5, 3)] = [
                    lambda: tail_psz.update(
                        g0=proj_partial(1, 0, tag="psz", last_cc=0))]
                post_slots[(1, 5, 3)] = (
                    lambda: tail_psz.update(
                        g1=proj_partial(1, 1, tag="psz", last_cc=0)))
                fill.extend([[]] * 6)
                for oc in range(NCC):
                    fill.append([("projA", oc)])
                    fill.append([("projB", oc)])
            else:
                # ---------------- tail ------------------------------------
                # Open remaining proj partials (cc 1-5, independent of the
                # straggler chunk 0) so the PE chews on them while the
                # recip chain for chunk 0 runs on DVE/DMA.
                tail_psz["g2"] = proj_partial(1, 2, tag="ps", last_cc=0)
                tail_psz["g3"] = proj_partial(1, 3, tag="ps", last_cc=0)
                # chunk 0 (heads 0,1) fast-path normalize: recip on the
                # [16,64] r_acc rows, broadcast via selector matmul into
                # PSUM (u's banks are free after the evac), y *= psR.
                r12_5 = rpool.tile([16, 64], BF16, tag="r125", name="r125")
                with nc.allow_low_precision(reason="bf16 recip of sums"):
                    nc.vector.reciprocal(r12_5[:], r_acc[1][0:16, :])
                nc.gpsimd.dma_start(r12pad[0:1, :], r12_5[:])
                psR = pu_pool.tile([P, N], F32, tag="u", name="psR5")
                nc.tensor.matmul(psR[:, 0:512], sel_sb[:], r12pad[:, 0:512],
                                 start=True, stop=True)
                nc.tensor.matmul(psR[:, 512:1024], sel_sb[:],
                                 r12pad[:, 512:1024], start=True, stop=True)
                ys = 0 * N + nh * 512
                nc.vector.tensor_mul(y_sb[0:64, ys: ys + 512],
                                     y_sb[0:64, ys: ys + 512],
                                     psR[0:64, 0:512])
                nc.vector.tensor_mul(y_sb[64:128, ys: ys + 512],
                                     y_sb[64:128, ys: ys + 512],
                                     psR[64:128, 512:1024])
                # close the pre-opened groups; the last two run whole on
                # slots freed by psR (pu) and g0's evacuation (psz)
                for oc in range(4):
                    proj_final(tail_psz[f"g{oc}"], 1, oc, last_cc=0)
                g4 = proj_partial(1, 4, tag="u", pool=pu_pool, last_cc=0,
                                  bufs=1)
                proj_final(g4, 1, 4, last_cc=0)
                g5 = proj_partial(1, 5, tag="psz", last_cc=0)
                proj_final(g5, 1, 5, last_cc=0)
    return nc


_NC_CACHE = None


def _get_nc():
    global _NC_CACHE
    if _NC_CACHE is None:
        _NC_CACHE = build_nc()
        _NC_CACHE.finalize()   # Bacc: runs wait legalization + reg alloc
    return _NC_CACHE


def prep_inputs(x, rel_pos_bias, qkv_w, proj_w, proj_b):
    """Host-side (free) layout transforms -> per-core in_maps."""
    import ml_dtypes

    B = x.shape[0]
    W = np.array(qkv_w, dtype=np.float32)
    W[:C] *= SCALE  # fold q scaling into weights
    qkwT_h = (W[:2 * C].T.reshape(NCC, P, NOC_QK, P)
              .transpose(1, 2, 0, 3).reshape(P, NOC_QK * NCC * P)
              .astype(ml_dtypes.bfloat16))
    wvT_h = (W[2 * C:].T.reshape(NCC, P, C)
             .transpose(1, 0, 2).reshape(P, NCC * C).astype(ml_dtypes.bfloat16))
    pwT_h = (np.asarray(proj_w, np.float32).T.reshape(NCC, P, NCC, P)
             .transpose(1, 2, 0, 3).reshape(P, NCC * NCC * P)
             .astype(ml_dtypes.bfloat16))
    pb_h = np.asarray(proj_b, np.float32).reshape(NCC, P).T.copy()
    # exp(bias) interleaved per (hp, t): [hp, mp, nh, t, hs, s, c]
    # (hs = head A/B within the pair, j = 2t+s is the m-chunk)
    eb = np.exp(np.asarray(rel_pos_bias, np.float32)[0])          # [H, n, m]
    ebT = eb.transpose(0, 2, 1)                                   # [H, m, n]
    eb5 = ebT.reshape(NCC, 2, 4, 2, P, 2, 512)  # hp, hs, t, s, mp, nh, c
    expBT_h = np.ascontiguousarray(
        eb5.transpose(0, 4, 5, 2, 1, 3, 6)      # hp, mp, nh, t, hs, s, c
        .reshape(NCC, P, 2 * 4 * 2048)).astype(ml_dtypes.bfloat16)
    in_maps = []
    for b in range(B):
        xT_h = np.ascontiguousarray(
            np.asarray(x[b], np.float32).T.reshape(NCC, P, N)
            .transpose(1, 0, 2).reshape(P, NCC * N).astype(ml_dtypes.bfloat16))
        in_maps.append({
            "xT": xT_h, "qkwT": qkwT_h, "wvT": wvT_h,
            "expBT": expBT_h, "pwT": pwT_h, "pb": pb_h,
        })
    return in_maps


def postprocess(zT_arr):
    """[128, 6*1024] zT -> [1024, 768] output for one batch."""
    return (np.asarray(zT_arr, np.float32).reshape(P, NCC, N)
            .transpose(1, 0, 2).reshape(C, N).T.copy())


def kernel(x, rel_pos_bias, qkv_w, proj_w, proj_b):
    from concourse.bass_utils import run_bass_kernel_spmd

    nc = _get_nc()
    in_maps = prep_inputs(x, rel_pos_bias, qkv_w, proj_w, proj_b)
    res = run_bass_kernel_spmd(nc, in_maps, list(range(len(in_maps))))
    out = np.stack([postprocess(r["zT"]) for r in res.results])
    return out.astype(np.float32)


# revision 23
# speedup vs baseline: 1.0484x; 1.0484x over previous
"""Trainium2 Bass kernel for multi-head attention with relative position bias.

Problem: x[8,1024,768] -> qkv proj -> 12-head attention (+rel_pos_bias,
softmax) -> out proj.  Sharding: pure batch-parallel, 1 batch per core on
8 cores, zero collectives; weights/bias replicated.

v3 (on top of v2's 128x128-tile-mode / padded-K scores / [v|ones] attn@v):
  - es/eb/p merged per t: ONE [128,2048] exp-bias tile per (nh,hp,t)
    (host interleaves A|B heads), ONE DVE multiply per t instead of two.
  - v laid out at stride 66 (dims 0:64, ones col 64, pad col 65) so the
    v' evacuation is 4B-aligned; one [128,768] psum chain per m-chunk.
  - sums land in recip-layout tiles r_acc[16c:16c+16, 64] directly from
    stg row 64 (no sums_row bounce hop).
  - DMA order: x/wv + qkw pairs 0-2 upfront; pairs 3-5 and pw dripped
    during attention so the first expB tiles arrive ~10us earlier.
  - nh1 chunks processed [1,2,3,4,5,0]; chunks 1-5 normalized inside
    pos5's t-slots; 4 of 6 final proj groups pre-opened as cc1-5
    partials so the tail is ~6 matmuls + short recip chain.

Per-core layout (host pre-transposes, free):
  - xT      [128, 6(cc) x 1024]     bf16  x[b].T chunked on contraction dim
  - qkwT    [128, 12(oc)x6(cc)x128] bf16  (q,k weights).T, q pre-scaled
  - wvT     [128, 6(cc) x 768]      bf16  v weights .T
  - expBT   [6(hp), 128, 2(nh)x4(t)x2048]  bf16  exp(bias) interleaved A|B
  - pwT     [128, 6(oc)x6(cc)x128]  bf16  proj_w.T
  - pb      [128, 6]                fp32  proj_b per o-chunk
  out zT    [128, 6(oc) x 1024]     bf16  z.T
"""

import numpy as np
from contextlib import ExitStack

import concourse.bass as bass
import concourse.mybir as mybir
from concourse import bacc
from concourse import tile

F32 = mybir.dt.float32
BF16 = mybir.dt.bfloat16

P = 128
N = 1024          # sequence
C = 768           # dim
H = 12            # heads
DH = 64           # head dim
NCC = 6           # contraction chunks (768/128)
NOC_QK = 12       # q,k output chunks (1536/128)
NM = 8            # seq chunks (1024/128)
VS = 66           # v group stride: [v 0:64 | ones 64 | pad 65]
SCALE = DH ** -0.5
EXPF = mybir.ActivationFunctionType.Exp


def build_nc():
    nc = bacc.Bacc(None, target_bir_lowering=False, debug=False)
    xT = nc.declare_dram_parameter("xT", [P, NCC * N], BF16, isOutput=False)
    qkwT = nc.declare_dram_parameter("qkwT", [P, NOC_QK * NCC * P], BF16, isOutput=False)
    wvT = nc.declare_dram_parameter("wvT", [P, NCC * C], BF16, isOutput=False)
    expBT = nc.declare_dram_parameter("expBT", [NCC, P, 2 * 4 * 2048], BF16, isOutput=False)
    pwT = nc.declare_dram_parameter("pwT", [P, NCC * NCC * P], BF16, isOutput=False)
    pb = nc.declare_dram_parameter("pb", [P, NCC], F32, isOutput=False)
    zT = nc.declare_dram_parameter("zT", [P, NCC * N], BF16, isOutput=True)

    with tile.TileContext(nc) as tc, ExitStack() as ctx:
        big = ctx.enter_context(tc.tile_pool(name="big", bufs=1))
        ebpool = ctx.enter_context(tc.tile_pool(name="eb", bufs=4))
        espool = ctx.enter_context(tc.tile_pool(name="es", bufs=3))
        ppool = ctx.enter_context(tc.tile_pool(name="p", bufs=3))
        stgpool = ctx.enter_context(tc.tile_pool(name="stg", bufs=3))
        rpool = ctx.enter_context(tc.tile_pool(name="r", bufs=1))
        drampool = ctx.enter_context(tc.tile_pool(name="dram", bufs=1, space="DRAM"))
        ps_pool = ctx.enter_context(tc.tile_pool(name="psum", bufs=2, space="PSUM"))
        pu_pool = ctx.enter_context(tc.tile_pool(name="psum_u", bufs=1, space="PSUM"))

        # persistent SBUF tensors
        q_sb = big.tile([P, NCC * N], BF16, tag="q")            # 12KB/part
        kpad_sb = big.tile([P, NOC_QK * N], BF16, tag="kpad")   # 24KB
        v_sb = big.tile([P, NM * H * VS], BF16, tag="vz")       # 12.4KB
        x_sb = big.tile([P, NCC * N], BF16, tag="xr")           # 12KB
        y_sb = big.tile([P, NCC * N], BF16, tag="y")            # 12KB
        qkw_sb = big.tile([P, NOC_QK * NCC * P], BF16, tag="qkw")  # 18KB
        wv_sb = big.tile([P, NCC * C], BF16, tag="wv")          # (shared w/ pw: 18KB)
        pb_sb = big.tile([P, NCC], F32, tag="pbt")

        # ---------------- phase 1 lead-in ---------------------------------
        # DMA priority: x + wv as single large transfers (v'/qk gate on
        # them), then qkw pairs 0-2, pb.  qkw pairs 3-5 and pw are emitted
        # later, inside the attention stream, so the first expB tiles
        # aren't stuck behind 7MB of queue.
        for cc in range(NCC):
            nc.sync.dma_start(x_sb[:, cc * N:(cc + 1) * N], xT[:, cc * N:(cc + 1) * N])
            nc.sync.dma_start(wv_sb[:, cc * C:(cc + 1) * C], wvT[:, cc * C:(cc + 1) * C])

        def load_qkw_pair(hp):
            for oc in (NCC + hp, hp):
                nc.sync.dma_start(qkw_sb[:, oc * NCC * P:(oc + 1) * NCC * P],
                                  qkwT[:, oc * NCC * P:(oc + 1) * NCC * P])

        for hp_ in range(3):
            load_qkw_pair(hp_)
        nc.sync.dma_start(pb_sb[:], pb[:])

        # zero the pad halves of kpad (k evacs later fill the live halves);
        # gpsimd is otherwise idle, keep this off the DVE critical path
        nc.gpsimd.memset(kpad_sb[:], 0.0)

        # ones column at 64 (so attention row-sums land on PSUM partition 64
        # and the v' evacuation stays 4B-aligned); col 65 is dead padding
        v_view = v_sb[:].rearrange("p (k d) -> p k d", d=VS)
        nc.vector.memset(v_view[:, :, 64:65], 1.0)
        nc.gpsimd.memset(v_view[:, :, 65:66], 0.0)

        # selector for the tail normalize: bcast row 0 via K=65 matmul
        # (tile mode stays 128x128); r12pad holds the reciprocal row with
        # zeroed pad rows (so the K=65 contraction sees no garbage)
        sel_sb = big.tile([65, P], BF16, tag="sel")
        nc.gpsimd.memset(sel_sb[:], 0.0)
        nc.vector.memset(sel_sb[0:1, :], 1.0)
        r12pad = big.tile([65, N], BF16, tag="r12pad")
        nc.gpsimd.memset(r12pad[:], 0.0)

        # per-nh recip-layout sums accumulators: chunk c's sums live at
        # rows [16c, 16c+16) x 64 (written straight from stg row 64).
        # memset so the full-tile reciprocal below never sees raw SBUF.
        r_acc = [big.tile([96, 64], BF16, tag=f"racc{i}", name=f"r_acc{i}")
                 for i in range(2)]
        nc.gpsimd.memset(r_acc[0][:], 1.0)
        nc.gpsimd.memset(r_acc[1][:], 1.0)

        # v projection (lead-in; attention needs all of v')
        for j in range(NM):
            psv = ps_pool.tile([P, C], F32, tag="ps", bufs=2, name=f"psv_{j}")
            for cc in range(NCC):
                lhs = x_sb[:, cc * N + j * P: cc * N + (j + 1) * P]
                nc.tensor.matmul(psv[:, 0:512], lhs, wv_sb[:, cc * C: cc * C + 512],
                                 start=(cc == 0), stop=(cc == NCC - 1))
                nc.tensor.matmul(psv[:, 512:768], lhs, wv_sb[:, cc * C + 512: (cc + 1) * C],
                                 start=(cc == 0), stop=(cc == NCC - 1))
            nc.vector.tensor_copy(
                v_view[:, j * H:(j + 1) * H, 0:64],
                psv[:].rearrange("p (h d) -> p h d", d=64))

        def qk_group(oc, qnh):
            ps = ps_pool.tile([P, 512], F32, tag="psz", bufs=2,
                              name=f"psqk_{oc}_{qnh}")
            for cc in range(NCC):
                nc.tensor.matmul(
                    ps[:],
                    qkw_sb[:, (oc * NCC + cc) * P:(oc * NCC + cc + 1) * P],
                    x_sb[:, cc * N + qnh * 512: cc * N + qnh * 512 + 512],
                    start=(cc == 0), stop=(cc == NCC - 1),
                )
            if oc < NCC:  # q chunk: both heads stacked, single copy
                nc.vector.tensor_copy(
                    q_sb[:, oc * N + qnh * 512: oc * N + qnh * 512 + 512], ps[:])
            else:         # k chunk: split into zero-padded A/B regions
                hp = oc - NCC
                nc.vector.tensor_copy(
                    kpad_sb[0:64, hp * N + qnh * 512: hp * N + qnh * 512 + 512],
                    ps[0:64, :])
                nc.vector.tensor_copy(
                    kpad_sb[64:128, oc * N + qnh * 512: oc * N + qnh * 512 + 512],
                    ps[64:128, :])

        def proj_accum(psz, pnh, oc, ccs, first):
            for cc in ccs:
                nc.tensor.matmul(
                    psz[:],
                    pw_sb[:, (oc * NCC + cc) * P:(oc * NCC + cc + 1) * P],
                    y_sb[:, cc * N + pnh * 512: cc * N + pnh * 512 + 512],
                    start=first, stop=False,
                )
                first = False

        def proj_partial(pnh, oc, tag="psz", last_cc=0, pool=None, bufs=2):
            """Open a proj accumulation over all cc except last_cc."""
            psz = (pool or ps_pool).tile([P, 512], F32, tag=tag, bufs=bufs,
                                         name=f"psz_{pnh}_{oc}")
            proj_accum(psz, pnh, oc,
                       [cc for cc in range(NCC) if cc != last_cc], True)
            return psz

        def proj_final(psz, pnh, oc, last_cc=0):
            nc.tensor.matmul(
                psz[:],
                pw_sb[:, (oc * NCC + last_cc) * P:(oc * NCC + last_cc + 1) * P],
                y_sb[:, last_cc * N + pnh * 512: last_cc * N + pnh * 512 + 512],
                start=False, stop=True,
            )
            nc.vector.tensor_scalar_add(
                zT_sb[:, oc * N + pnh * 512: oc * N + pnh * 512 + 512],
                psz[:], pb_sb[:, oc:oc + 1])
            nc.sync.dma_start(
                zT[:, oc * N + pnh * 512: oc * N + pnh * 512 + 512],
                zT_sb[:, oc * N + pnh * 512: oc * N + pnh * 512 + 512])

        def proj_group(pnh, oc):
            psz = proj_partial(pnh, oc, last_cc=0)
            proj_final(psz, pnh, oc, last_cc=0)

        proj_open = {}

        def drip(items):
            for kind in items:
                if kind[0] == "projA":    # open psz, accumulate cc 1-3
                    oc = kind[1]
                    psz = ps_pool.tile([P, 512], F32, tag="psz", bufs=2,
                                       name=f"psz_0_{oc}")
                    proj_accum(psz, 0, oc, [1, 2, 3], True)
                    proj_open[oc] = psz
                elif kind[0] == "projB":  # cc 4,5 then close with cc0
                    oc = kind[1]
                    proj_accum(proj_open[oc], 0, oc, [4, 5], False)
                    proj_final(proj_open.pop(oc), 0, oc, last_cc=0)
                else:
                    qk_group(kind[1], kind[2])

        from collections import deque
        # pair 0's qnh0 k/q in the lead-in; the rest dripped into the
        # attention stream just ahead of need (q qnh1 halves are only
        # read in nh1, so they fill nh0's late, otherwise-idle slots)
        qk_group(NCC, 0)
        qk_group(0, 0)
        K_ = NCC  # k chunk oc offset
        fill = deque([
            [("qk", K_ + 0, 1), ("qk", K_ + 1, 0)],   # s0
            [("qk", K_ + 1, 1)], [("qk", 1, 0)],      # s1-2
            [("qk", K_ + 2, 0)], [("qk", K_ + 2, 1)], [("qk", 2, 0)],
            [("qk", K_ + 3, 0)], [("qk", K_ + 3, 1)], [("qk", 3, 0)],
            [("qk", K_ + 4, 0)], [("qk", K_ + 4, 1)], [("qk", 4, 0)],
            [("qk", K_ + 5, 0)], [("qk", K_ + 5, 1)], [("qk", 5, 0)],
            [("qk", 1, 1)], [("qk", 2, 1)], [("qk", 3, 1)],
            [("qk", 4, 1)], [("qk", 5, 1)], [("qk", 0, 1)],            # s15-20
        ])

        # ---------------- phase 2: attention, n-half outer ----------------
        pw_sb = big.tile([P, NCC * NCC * P], BF16, tag="wv")  # wv dead after phase 1
        zT_sb = big.tile([P, NCC * N], BF16, tag="zt")

        R_sb = big.tile([P, NCC * N], BF16, tag="xr")   # reuse x slot

        # Deferred weight DMAs, dripped at chunk starts so they don't
        # delay the first expB tiles in the DMA queues.
        def load_pw_half(half):
            hw = NCC * NCC * P // 2
            nc.sync.dma_start(pw_sb[:, half * hw:(half + 1) * hw],
                              pwT[:, half * hw:(half + 1) * hw])

        chunk_dmas = {
            (0, 0): lambda: load_qkw_pair(3),
            (0, 1): lambda: load_qkw_pair(4),
            (0, 2): lambda: (load_qkw_pair(5), load_pw_half(0)),
            (0, 3): lambda: load_pw_half(1),
        }

        # Batched normalize for chunk range [c0, c1) of n-half nh, staged
        # across t-slots so each engine op is emitted only after its deps
        # retired (engine queues are FIFO; an op with pending deps blocks
        # its queue).  Stage A: reciprocal of r_acc rows + DRAM bounce out.
        # Stage B: broadcast back into R_sb.  Stage C: y *= R.
        def norm_stageA(nh, c0, c1):
            # DVE partition base must be 32-aligned: reciprocal the whole
            # 96-row tile (rows of absent chunks hold 1.0 from the memset),
            # then bounce only the live rows out through DRAM.
            nch = c1 - c0
            r12 = rpool.tile([96, 64], BF16, tag="r12", bufs=2,
                             name=f"r12_{nh}_{c0}")
            with nc.allow_low_precision(reason="bf16 recip of softmax sums"):
                nc.vector.reciprocal(r12[:], r_acc[nh][:])
            r_d = drampool.tile([16 * nch, 64], BF16, bufs=2,
                                name=f"r_d_{nh}_{c0}")
            # norm-chain DMAs ride the idle gpsimd queue: the sync queue
            # carries the expB stream and has ~10us head-of-line latency
            nc.sync.dma_start(r_d[:], r12[16 * c0:16 * c1, :])
            return r_d

        def norm_stageB(nh, r_d, c0, c1):
            nch = c1 - c0
            r_pairs = (r_d[:].rearrange("l n -> (l n)")
                       .rearrange("(c n) -> c n", n=N))
            for par in range(2):  # even heads -> parts 0:64, odd -> 64:128
                nc.sync.dma_start(
                    R_sb[64 * par: 64 * par + 64, :]
                    .rearrange("p (c n) -> p c n", n=N)
                    [:, c0:c1, nh * 512:nh * 512 + 512],
                    r_pairs[:, par * 512:par * 512 + 512][None, :, :]
                    .broadcast_to([64, nch, 512]),
                )

        def norm_stageC(nh, c0, c1):
            # per-chunk (512-wide) so no single DVE op piles onto the
            # queue ahead of the attention multiplies
            yh = (y_sb[:].rearrange("p (c n) -> p c n", n=N)
                  [:, c0:c1, nh * 512:(nh + 1) * 512])
            Rh = (R_sb[:].rearrange("p (c n) -> p c n", n=N)
                  [:, c0:c1, nh * 512:(nh + 1) * 512])
            nc.vector.tensor_mul(yh, yh, Rh)

        # staged-normalize / tail-prep slots: {(nh, pos, t): callable}
        norm_slots = {}
        post_slots = {}
        tail_psz = {}
        st = {}

        # nh0 chunks in order 0..5; nh1 in order 1..5,0 so the straggler
        # chunk is 0 and chunks [1,6) batch-normalize contiguously.
        for nh in range(2):
            order = list(range(NCC)) if nh == 0 else [1, 2, 3, 4, 5, 0]
            for pos, hp in enumerate(order):
                fn = chunk_dmas.pop((nh, pos), None)
                if fn is not None:
                    fn()
                hA, hB = 2 * hp, 2 * hp + 1
                u = pu_pool.tile([65, N], F32, tag="u", name=f"u_{nh}_{hp}")
                rhs_q = q_sb[:, hp * N + nh * 512: hp * N + nh * 512 + 512]
                for t in range(4):
                    for fn in norm_slots.pop((nh, pos, t), []):
                        fn()
                    ebAB = ebpool.tile([P, 2048], BF16, tag="eb")
                    nc.sync.dma_start(
                        ebAB[:], expBT[hp, :, nh * 8192 + t * 2048:
                                       nh * 8192 + (t + 1) * 2048])
                    psA = ps_pool.tile([P, N], F32, tag="ps", bufs=2,
                                       name=f"psA_{nh}_{hp}_{t}")
                    psB = ps_pool.tile([P, N], F32, tag="ps", bufs=2,
                                       name=f"psB_{nh}_{hp}_{t}")
                    for s in range(2):
                        j = 2 * t + s
                        kA = kpad_sb[:, hp * N + j * P: hp * N + (j + 1) * P]
                        kB = kpad_sb[:, (NCC + hp) * N + j * P: (NCC + hp) * N + (j + 1) * P]
                        nc.tensor.matmul(psA[:, s * 512:(s + 1) * 512], kA, rhs_q,
                                         start=True, stop=True)
                        nc.tensor.matmul(psB[:, s * 512:(s + 1) * 512], kB, rhs_q,
                                         start=True, stop=True)
                    # drip deferred matmul groups into the stream
                    if fill:
                        drip(fill.popleft())
                    fn = post_slots.pop((nh, pos, t), None)
                    if fn is not None:
                        fn()
                    esAB = espool.tile([P, 2048], BF16, tag="esAB")
                    nc.scalar.activation(esAB[:, 0:1024], psA[:], EXPF)
                    nc.scalar.activation(esAB[:, 1024:2048], psB[:], EXPF)
                    pAB = ppool.tile([P, 2048], BF16, tag="pAB")
                    nc.vector.tensor_mul(pAB[:], esAB[:], ebAB[:])
                    for s in range(2):
                        j = 2 * t + s
                        vA = v_sb[:, (j * H + hA) * VS: (j * H + hA) * VS + 65]
                        vB = v_sb[:, (j * H + hB) * VS: (j * H + hB) * VS + 65]
                        nc.tensor.matmul(u[:, 0:512], vA, pAB[:, s * 512:(s + 1) * 512],
                                         start=(j == 0), stop=(j == NM - 1),
                                         skip_group_check=True)
                        nc.tensor.matmul(u[:, 512:1024], vB,
                                         pAB[:, 1024 + s * 512: 1024 + (s + 1) * 512],
                                         start=(j == 0), stop=(j == NM - 1),
                                         skip_group_check=True)
                # evacuate u via one copy; DMAs scatter y halves + sums
                # (u rows 0:64 = head dims, row 64 = sums)
                ys = hp * N + nh * 512
                stg = stgpool.tile([65, N], BF16, tag="stg")
                nc.vector.tensor_copy(stg[:], u[:])
                nc.sync.dma_start(y_sb[0:64, ys: ys + 512], stg[0:64, 0:512])
                nc.sync.dma_start(y_sb[64:128, ys: ys + 512], stg[0:64, 512:1024])
                nc.sync.dma_start(
                    r_acc[nh][16 * hp:16 * hp + 16, :],
                    stg[64:65, :])
                if nh == 0 and pos == NCC - 2:
                    # nh0 chunks 0-4 normalize staged into pos5's t-slots
                    norm_slots[(0, NCC - 1, 1)] = [
                        lambda: st.update(rd0=norm_stageA(0, 0, NCC - 1))]
                    norm_slots[(0, NCC - 1, 3)] = [
                        lambda: norm_stageB(0, st["rd0"], 0, NCC - 1)]
            if nh == 0:
                # nh0 normalize: batched recip/bounce done in pos5's slots;
                # the per-chunk y-multiplies spread one per slot across
                # nh1's first iterations (needed by slot 30's proj drips).
                # nh0 chunk 5's own bounce rides nh1-pos0's slots.
                norm_slots[(1, 0, 0)] = [lambda: norm_stageC(0, 0, 1)]
                norm_slots[(1, 0, 1)] = [
                    lambda: st.update(rd05=norm_stageA(0, NCC - 1, NCC)),
                    lambda: norm_stageC(0, 1, 2)]
                norm_slots[(1, 0, 2)] = [lambda: norm_stageC(0, 2, 3)]
                norm_slots[(1, 0, 3)] = [
                    lambda: norm_stageB(0, st["rd05"], NCC - 1, NCC),
                    lambda: norm_stageC(0, 3, 4)]
                norm_slots[(1, 1, 0)] = [lambda: norm_stageC(0, 4, 5)]
                norm_slots[(1, 1, 1)] = [lambda: norm_stageC(0, NCC - 1, NCC)]
                # nh1 normalize pipelined in two waves (chunks 1-4 during
                # pos4, chunk 5 during pos5) so the proj(1,*) cc1-5
                # partials can pre-run before straggler chunk 0 finishes
                norm_slots[(1, 4, 0)] = [
                    lambda: st.update(rd14=norm_stageA(1, 1, 5))]
                norm_slots[(1, 4, 1)] = [
                    lambda: norm_stageB(1, st["rd14"], 1, 5)]
                norm_slots[(1, 4, 2)] = [lambda: norm_stageC(1, 1, 2)]
                norm_slots[(1, 4, 3)] = [lambda: norm_stageC(1, 2, 3)]
                norm_slots[(1, 5, 0)] = [
                    lambda: norm_stageC(1, 3, 4),
                    lambda: st.update(rd15=norm_stageA(1, 5, 6))]
                norm_slots[(1, 5, 1)] = [
                    lambda: norm_stageC(1, 4, 5),
                    lambda: norm_stageB(1, st["rd15"], 5, 6)]
                norm_slots[(1, 5, 2)] = [lambda: norm_stageC(1, 5, 6)]
                norm_slots[(1, 5, 3)] = [
                    lambda: tail_psz.update(
                        g0=proj_partial(1, 0, tag="psz", last_cc=0))]
                post_slots[(1, 5, 3)] = (
                    lambda: tail_psz.update(
                        g1=proj_partial(1, 1, tag="psz", last_cc=0)))
                fill.extend([[]] * 6)
                for oc in range(NCC):
                    fill.append([("projA", oc)])
                    fill.append([("projB", oc)])
            else:
                # ---------------- tail ------------------------------------
                # Open remaining proj partials (cc 1-5, independent of the
                # straggler chunk 0) so the PE chews on them while the
                # recip chain for chunk 0 runs on DVE/DMA.
                tail_psz["g2"] = proj_partial(1, 2, tag="ps", last_cc=0)
                tail_psz["g3"] = proj_partial(1, 3, tag="ps", last_cc=0)
                # chunk 0 (heads 0,1) fast-path normalize: recip on the
                # [16,64] r_acc rows, broadcast via selector matmul into
                # PSUM (u's banks are free after the evac), y *= psR.
                r12_5 = rpool.tile([16, 64], BF16, tag="r125", name="r125")
                with nc.allow_low_precision(reason="bf16 recip of sums"):
                    nc.vector.reciprocal(r12_5[:], r_acc[1][0:16, :])
                nc.sync.dma_start(r12pad[0:1, :], r12_5[:])
                psR = pu_pool.tile([P, N], F32, tag="u", name="psR5")
                nc.tensor.matmul(psR[:, 0:512], sel_sb[:], r12pad[:, 0:512],
                                 start=True, stop=True)
                nc.tensor.matmul(psR[:, 512:1024], sel_sb[:],
                                 r12pad[:, 512:1024], start=True, stop=True)
                ys = 0 * N + nh * 512
                nc.vector.tensor_mul(y_sb[0:64, ys: ys + 512],
                                     y_sb[0:64, ys: ys + 512],
                                     psR[0:64, 0:512])
                nc.vector.tensor_mul(y_sb[64:128, ys: ys + 512],
                                     y_sb[64:128, ys: ys + 512],
                                     psR[64:128, 512:1024])
                # close the pre-opened groups; the last two run whole on
                # slots freed by psR (pu) and g0's evacuation (psz)
                for oc in range(4):
                    proj_final(tail_psz[f"g{oc}"], 1, oc, last_cc=0)
                g4 = proj_partial(1, 4, tag="u", pool=pu_pool, last_cc=0,
                                  bufs=1)
                proj_final(g4, 1, 4, last_cc=0)
                g5 = proj_partial(1, 5, tag="psz", last_cc=0)
                proj_final(g5, 1, 5, last_cc=0)
    return nc


_NC_CACHE = None


def _get_nc():
    global _NC_CACHE
    if _NC_CACHE is None:
        _NC_CACHE = build_nc()
        _NC_CACHE.finalize()   # Bacc: runs wait legalization + reg alloc
    return _NC_CACHE


def prep_inputs(x, rel_pos_bias, qkv_w, proj_w, proj_b):
    """Host-side (free) layout transforms -> per-core in_maps."""
    import ml_dtypes

    B = x.shape[0]
    W = np.array(qkv_w, dtype=np.float32)
    W[:C] *= SCALE  # fold q scaling into weights
    qkwT_h = (W[:2 * C].T.reshape(NCC, P, NOC_QK, P)
              .transpose(1, 2, 0, 3).reshape(P, NOC_QK * NCC * P)
              .astype(ml_dtypes.bfloat16))
    wvT_h = (W[2 * C:].T.reshape(NCC, P, C)
             .transpose(1, 0, 2).reshape(P, NCC * C).astype(ml_dtypes.bfloat16))
    pwT_h = (np.asarray(proj_w, np.float32).T.reshape(NCC, P, NCC, P)
             .transpose(1, 2, 0, 3).reshape(P, NCC * NCC * P)
             .astype(ml_dtypes.bfloat16))
    pb_h = np.asarray(proj_b, np.float32).reshape(NCC, P).T.copy()
    # exp(bias) interleaved per (hp, t): [hp, mp, nh, t, hs, s, c]
    # (hs = head A/B within the pair, j = 2t+s is the m-chunk)
    eb = np.exp(np.asarray(rel_pos_bias, np.float32)[0])          # [H, n, m]
    ebT = eb.transpose(0, 2, 1)                                   # [H, m, n]
    eb5 = ebT.reshape(NCC, 2, 4, 2, P, 2, 512)  # hp, hs, t, s, mp, nh, c
    expBT_h = np.ascontiguousarray(
        eb5.transpose(0, 4, 5, 2, 1, 3, 6)      # hp, mp, nh, t, hs, s, c
        .reshape(NCC, P, 2 * 4 * 2048)).astype(ml_dtypes.bfloat16)
    in_maps = []
    for b in range(B):
        xT_h = np.ascontiguousarray(
            np.asarray(x[b], np.float32).T.reshape(NCC, P, N)
            .transpose(1, 0, 2).reshape(P, NCC * N).astype(ml_dtypes.bfloat16))
        in_maps.append({
            "xT": xT_h, "qkwT": qkwT_h, "wvT": wvT_h,
            "expBT": expBT_h, "pwT": pwT_h, "pb": pb_h,
        })
    return in_maps


def postprocess(zT_arr):
    """[128, 6*1024] zT -> [1024, 768] output for one batch."""
    return (np.asarray(zT_arr, np.float32).reshape(P, NCC, N)
            .transpose(1, 0, 2).reshape(C, N).T.copy())


def kernel(x, rel_pos_bias, qkv_w, proj_w, proj_b):
    from concourse.bass_utils import run_bass_kernel_spmd

    nc = _get_nc()
    in_maps = prep_inputs(x, rel_pos_bias, qkv_w, proj_w, proj_b)
    res = run_bass_kernel_spmd(nc, in_maps, list(range(len(in_maps))))
    out = np.stack([postprocess(r["zT"]) for r in res.results])
    return out.astype(np.float32)


# revision 31
# speedup vs baseline: 1.0757x; 1.0260x over previous
"""Trainium2 Bass kernel for multi-head attention with relative position bias.

Problem: x[8,1024,768] -> qkv proj -> 12-head attention (+rel_pos_bias,
softmax) -> out proj.  Sharding: pure batch-parallel, 1 batch per core on
8 cores, zero collectives; weights/bias replicated.

v3 (on top of v2's 128x128-tile-mode / padded-K scores / [v|ones] attn@v):
  - es/eb/p merged per t: ONE [128,2048] exp-bias tile per (nh,hp,t)
    (host interleaves A|B heads), ONE DVE multiply per t instead of two.
  - v laid out at stride 66 (dims 0:64, ones col 64, pad col 65) so the
    v' evacuation is 4B-aligned; one [128,768] psum chain per m-chunk.
  - sums land in recip-layout tiles r_acc[16c:16c+16, 64] directly from
    stg row 64 (no sums_row bounce hop).
  - DMA order: x/wv + qkw pairs 0-2 upfront; pairs 3-5 and pw dripped
    during attention so the first expB tiles arrive ~10us earlier.
  - nh1 chunks processed [1,2,3,4,5,0]; chunks 1-5 normalized inside
    pos5's t-slots; 4 of 6 final proj groups pre-opened as cc1-5
    partials so the tail is ~6 matmuls + short recip chain.

Per-core layout (host pre-transposes, free):
  - xT      [128, 6(cc) x 1024]     bf16  x[b].T chunked on contraction dim
  - qkwT    [128, 12(oc)x6(cc)x128] bf16  (q,k weights).T, q pre-scaled
  - wvT     [128, 6(cc) x 768]      bf16  v weights .T
  - expBT   [6(hp), 128, 2(nh)x4(t)x2048]  bf16  exp(bias) interleaved A|B
  - pwT     [128, 6(oc)x6(cc)x128]  bf16  proj_w.T
  - pb      [128, 6]                fp32  proj_b per o-chunk
  out zT    [128, 6(oc) x 1024]     bf16  z.T
"""

import numpy as np
from contextlib import ExitStack

import concourse.bass as bass
import concourse.mybir as mybir
from concourse import bacc
from concourse import tile

F32 = mybir.dt.float32
BF16 = mybir.dt.bfloat16

P = 128
N = 1024          # sequence
C = 768           # dim
H = 12            # heads
DH = 64           # head dim
NCC = 6           # contraction chunks (768/128)
NOC_QK = 12       # q,k output chunks (1536/128)
NM = 8            # seq chunks (1024/128)
VS = 66           # v group stride: [v 0:64 | ones 64 | pad 65]
SCALE = DH ** -0.5
EXPF = mybir.ActivationFunctionType.Exp


def build_nc():
    nc = bacc.Bacc(None, target_bir_lowering=False, debug=False)
    xT = nc.declare_dram_parameter("xT", [P, NCC * N], BF16, isOutput=False)
    qkwT = nc.declare_dram_parameter("qkwT", [P, NOC_QK * NCC * P], BF16, isOutput=False)
    wvT = nc.declare_dram_parameter("wvT", [P, NCC * C], BF16, isOutput=False)
    expBT = nc.declare_dram_parameter("expBT", [NCC, P, 2 * 4 * 2048], BF16, isOutput=False)
    pwT = nc.declare_dram_parameter("pwT", [P, NCC * NCC * P], BF16, isOutput=False)
    pb = nc.declare_dram_parameter("pb", [P, NCC], F32, isOutput=False)
    zT = nc.declare_dram_parameter("zT", [P, NCC * N], BF16, isOutput=True)

    with tile.TileContext(nc) as tc, ExitStack() as ctx:
        big = ctx.enter_context(tc.tile_pool(name="big", bufs=1))
        ebpool = ctx.enter_context(tc.tile_pool(name="eb", bufs=4))
        espool = ctx.enter_context(tc.tile_pool(name="es", bufs=3))
        ppool = ctx.enter_context(tc.tile_pool(name="p", bufs=3))
        stgpool = ctx.enter_context(tc.tile_pool(name="stg", bufs=3))
        rpool = ctx.enter_context(tc.tile_pool(name="r", bufs=1))
        drampool = ctx.enter_context(tc.tile_pool(name="dram", bufs=1, space="DRAM"))
        ps_pool = ctx.enter_context(tc.tile_pool(name="psum", bufs=2, space="PSUM"))
        pu_pool = ctx.enter_context(tc.tile_pool(name="psum_u", bufs=1, space="PSUM"))

        # persistent SBUF tensors
        q_sb = big.tile([P, NCC * N], BF16, tag="q")            # 12KB/part
        kpad_sb = big.tile([P, NOC_QK * N], BF16, tag="kpad")   # 24KB
        v_sb = big.tile([P, NM * H * VS], BF16, tag="vz")       # 12.4KB
        x_sb = big.tile([P, NCC * N], BF16, tag="xr")           # 12KB
        y_sb = big.tile([P, NCC * N], BF16, tag="y")            # 12KB
        qkw_sb = big.tile([P, NOC_QK * NCC * P], BF16, tag="qkw")  # 18KB
        wv_sb = big.tile([P, NCC * C], BF16, tag="wv")          # (shared w/ pw: 18KB)
        pb_sb = big.tile([P, NCC], F32, tag="pbt")

        # ---------------- phase 1 lead-in ---------------------------------
        # DMA priority: x + wv as single large transfers (v'/qk gate on
        # them), then qkw pairs 0-2, pb.  qkw pairs 3-5 and pw are emitted
        # later, inside the attention stream, so the first expB tiles
        # aren't stuck behind 7MB of queue.
        for cc in range(NCC):
            nc.sync.dma_start(x_sb[:, cc * N:(cc + 1) * N], xT[:, cc * N:(cc + 1) * N])
            nc.sync.dma_start(wv_sb[:, cc * C:(cc + 1) * C], wvT[:, cc * C:(cc + 1) * C])

        def load_qkw_pair(hp):
            for oc in (NCC + hp, hp):
                nc.sync.dma_start(qkw_sb[:, oc * NCC * P:(oc + 1) * NCC * P],
                                  qkwT[:, oc * NCC * P:(oc + 1) * NCC * P])

        for hp_ in range(3):
            load_qkw_pair(hp_)
        nc.sync.dma_start(pb_sb[:], pb[:])

        # zero the pad halves of kpad (k evacs later fill the live halves);
        # gpsimd is otherwise idle, keep this off the DVE critical path
        nc.gpsimd.memset(kpad_sb[:], 0.0)

        # ones column at 64 (so attention row-sums land on PSUM partition 64
        # and the v' evacuation stays 4B-aligned); col 65 is dead padding
        v_view = v_sb[:].rearrange("p (k d) -> p k d", d=VS)
        nc.vector.memset(v_view[:, :, 64:65], 1.0)
        nc.gpsimd.memset(v_view[:, :, 65:66], 0.0)

        # selector for the tail normalize: bcast row 0 via K=65 matmul
        # (tile mode stays 128x128); r12pad holds the reciprocal row with
        # zeroed pad rows (so the K=65 contraction sees no garbage)
        sel_sb = big.tile([65, P], BF16, tag="sel")
        nc.gpsimd.memset(sel_sb[:], 0.0)
        nc.vector.memset(sel_sb[0:1, :], 1.0)
        r12pad = big.tile([65, N], BF16, tag="r12pad")
        nc.gpsimd.memset(r12pad[:], 0.0)

        # per-nh recip-layout sums accumulators: chunk c's sums live at
        # rows [16c, 16c+16) x 64 (written straight from stg row 64).
        # memset so the full-tile reciprocal below never sees raw SBUF.
        r_acc = [big.tile([96, 64], BF16, tag=f"racc{i}", name=f"r_acc{i}")
                 for i in range(2)]
        nc.gpsimd.memset(r_acc[0][:], 1.0)
        nc.gpsimd.memset(r_acc[1][:], 1.0)

        # v projection: j-chunk halves on 1-bank psz tiles so chunks 2-7
        # can drip into the first attention iterations (chunk j is needed
        # by attn@v at t = j//2 of the first head pair)
        def vproj(j, lo):
            w0, w1 = (0, 512) if lo == 0 else (512, 768)
            psv = ps_pool.tile([P, w1 - w0], F32, tag="psz", bufs=2,
                               name=f"psv_{j}_{lo}")
            for cc in range(NCC):
                lhs = x_sb[:, cc * N + j * P: cc * N + (j + 1) * P]
                nc.tensor.matmul(psv[:], lhs, wv_sb[:, cc * C + w0: cc * C + w1],
                                 start=(cc == 0), stop=(cc == NCC - 1))
            h0 = lo * 8
            nc.vector.tensor_copy(
                v_view[:, j * H + h0: j * H + h0 + (w1 - w0) // 64, 0:64],
                psv[:].rearrange("p (h d) -> p h d", d=64))

        qk_open = {}

        def qk_part(oc, qnh, ccs, close):
            """Half of a q/k projection group; `close` evacuates."""
            key = (oc, qnh)
            if key not in qk_open:
                qk_open[key] = ps_pool.tile([P, 512], F32, tag="psz", bufs=2,
                                            name=f"psqk_{oc}_{qnh}")
            ps = qk_open[key]
            for i, cc in enumerate(ccs):
                nc.tensor.matmul(
                    ps[:],
                    qkw_sb[:, (oc * NCC + cc) * P:(oc * NCC + cc + 1) * P],
                    x_sb[:, cc * N + qnh * 512: cc * N + qnh * 512 + 512],
                    start=(ccs[0] == 0 and i == 0), stop=(close and cc == ccs[-1]),
                )
            if not close:
                return
            qk_open.pop(key)
            if oc < NCC:  # q chunk: both heads stacked, single copy
                nc.vector.tensor_copy(
                    q_sb[:, oc * N + qnh * 512: oc * N + qnh * 512 + 512], ps[:])
            else:         # k chunk: split into zero-padded A/B regions
                hp = oc - NCC
                nc.vector.tensor_copy(
                    kpad_sb[0:64, hp * N + qnh * 512: hp * N + qnh * 512 + 512],
                    ps[0:64, :])
                nc.vector.tensor_copy(
                    kpad_sb[64:128, oc * N + qnh * 512: oc * N + qnh * 512 + 512],
                    ps[64:128, :])

        def qk_group(oc, qnh):
            qk_part(oc, qnh, [0, 1, 2, 3, 4, 5], True)

        def proj_accum(psz, pnh, oc, ccs, first):
            for cc in ccs:
                nc.tensor.matmul(
                    psz[:],
                    pw_sb[:, (oc * NCC + cc) * P:(oc * NCC + cc + 1) * P],
                    y_sb[:, cc * N + pnh * 512: cc * N + pnh * 512 + 512],
                    start=first, stop=False,
                )
                first = False

        def proj_partial(pnh, oc, tag="psz", last_cc=0, pool=None, bufs=2):
            """Open a proj accumulation over all cc except last_cc."""
            psz = (pool or ps_pool).tile([P, 512], F32, tag=tag, bufs=bufs,
                                         name=f"psz_{pnh}_{oc}")
            proj_accum(psz, pnh, oc,
                       [cc for cc in range(NCC) if cc != last_cc], True)
            return psz

        def proj_final(psz, pnh, oc, last_cc=0):
            nc.tensor.matmul(
                psz[:],
                pw_sb[:, (oc * NCC + last_cc) * P:(oc * NCC + last_cc + 1) * P],
                y_sb[:, last_cc * N + pnh * 512: last_cc * N + pnh * 512 + 512],
                start=False, stop=True,
            )
            nc.vector.tensor_scalar_add(
                zT_sb[:, oc * N + pnh * 512: oc * N + pnh * 512 + 512],
                psz[:], pb_sb[:, oc:oc + 1])
            nc.sync.dma_start(
                zT[:, oc * N + pnh * 512: oc * N + pnh * 512 + 512],
                zT_sb[:, oc * N + pnh * 512: oc * N + pnh * 512 + 512])

        def proj_group(pnh, oc):
            psz = proj_partial(pnh, oc, last_cc=0)
            proj_final(psz, pnh, oc, last_cc=0)

        proj_open = {}

        def drip(items):
            for kind in items:
                if kind[0] == "projA":    # open psz, accumulate cc 1-3
                    oc = kind[1]
                    psz = ps_pool.tile([P, 512], F32, tag="psz", bufs=2,
                                       name=f"psz_0_{oc}")
                    proj_accum(psz, 0, oc, [1, 2, 3], True)
                    proj_open[oc] = psz
                elif kind[0] == "projB":  # cc 4,5 then close with cc0
                    oc = kind[1]
                    proj_accum(proj_open[oc], 0, oc, [4, 5], False)
                    proj_final(proj_open.pop(oc), 0, oc, last_cc=0)
                elif kind[0] == "vp":
                    vproj(kind[1], kind[2])
                elif kind[0] == "qka":
                    qk_part(kind[1], kind[2], [0, 1, 2], False)
                elif kind[0] == "qkb":
                    qk_part(kind[1], kind[2], [3, 4, 5], True)
                else:
                    qk_group(kind[1], kind[2])

        from collections import deque
        # qk pair-0 (qnh0) + v chunks 0,1 in the lead-in; v chunks 2-7 and
        # all other q/k groups drip into the attention stream just ahead
        # of need (q qnh1 halves are only read in nh1, so they fill nh0's
        # late, otherwise-idle slots) -- this starts the exp pipeline
        # ~14us earlier than a serial lead-in
        qk_group(NCC, 0)
        qk_group(0, 0)
        for j_ in (0, 1):
            vproj(j_, 0)
            vproj(j_, 1)
        K_ = NCC  # k chunk oc offset
        fill = deque([
            [("qk", K_ + 0, 1), ("vp", 2, 0), ("vp", 2, 1),
             ("vp", 3, 0), ("vp", 3, 1)],                               # s0
            [("vp", 4, 0), ("vp", 4, 1), ("vp", 5, 0), ("vp", 5, 1)],   # s1
            [("vp", 6, 0), ("vp", 6, 1), ("vp", 7, 0), ("vp", 7, 1)],   # s2
            [("qk", K_ + 1, 0), ("qk", 1, 0)],                          # s3
            [("qk", K_ + 1, 1)],                                        # s4
            [("qk", K_ + 2, 0)], [("qk", 2, 0)], [("qk", K_ + 2, 1)],   # s5-7
            [("qk", K_ + 3, 0)], [("qk", 3, 0)], [("qk", K_ + 3, 1)],   # s8-10
            [("qk", K_ + 4, 0)], [("qk", 4, 0)], [("qk", K_ + 4, 1)],   # s11-13
            [("qk", K_ + 5, 0)], [("qk", 5, 0)], [("qk", K_ + 5, 1)],   # s14-16
            [("qka", 1, 1)], [("qkb", 1, 1)],                           # s17-18
            [("qka", 2, 1)], [("qkb", 2, 1)],
            [("qka", 3, 1)], [("qkb", 3, 1)],
            [("qka", 4, 1)], [("qkb", 4, 1)],
            [("qka", 5, 1)], [("qkb", 5, 1)],
            [("qka", 0, 1)], [("qkb", 0, 1)],                           # s27-28
        ])

        # ---------------- phase 2: attention, n-half outer ----------------
        pw_sb = big.tile([P, NCC * NCC * P], BF16, tag="wv")  # wv dead after phase 1
        zT_sb = big.tile([P, NCC * N], BF16, tag="zt")

        R_sb = big.tile([P, NCC * N], BF16, tag="xr")   # reuse x slot

        # Deferred weight DMAs, dripped at chunk starts so they don't
        # delay the first expB tiles in the DMA queues.
        def load_pw_half(half):
            hw = NCC * NCC * P // 2
            nc.sync.dma_start(pw_sb[:, half * hw:(half + 1) * hw],
                              pwT[:, half * hw:(half + 1) * hw])

        chunk_dmas = {
            (0, 0): lambda: load_qkw_pair(3),
            (0, 1): lambda: load_qkw_pair(4),
            (0, 2): lambda: (load_qkw_pair(5), load_pw_half(0)),
            (0, 3): lambda: load_pw_half(1),
        }

        # Batched normalize for chunk range [c0, c1) of n-half nh, staged
        # across t-slots so each engine op is emitted only after its deps
        # retired (engine queues are FIFO; an op with pending deps blocks
        # its queue).  Stage A: reciprocal of r_acc rows + DRAM bounce out.
        # Stage B: broadcast back into R_sb.  Stage C: y *= R.
        def norm_stageA(nh, c0, c1):
            # DVE partition base must be 32-aligned: reciprocal the whole
            # 96-row tile (rows of absent chunks hold 1.0 from the memset),
            # then bounce only the live rows out through DRAM.
            nch = c1 - c0
            r12 = rpool.tile([96, 64], BF16, tag="r12", bufs=2,
                             name=f"r12_{nh}_{c0}")
            with nc.allow_low_precision(reason="bf16 recip of softmax sums"):
                nc.vector.reciprocal(r12[:], r_acc[nh][:])
            r_d = drampool.tile([16 * nch, 64], BF16, bufs=2,
                                name=f"r_d_{nh}_{c0}")
            # norm-chain DMAs ride the idle gpsimd queue: the sync queue
            # carries the expB stream and has ~10us head-of-line latency
            nc.sync.dma_start(r_d[:], r12[16 * c0:16 * c1, :])
            return r_d

        def norm_stageB(nh, r_d, c0, c1):
            nch = c1 - c0
            r_pairs = (r_d[:].rearrange("l n -> (l n)")
                       .rearrange("(c n) -> c n", n=N))
            for par in range(2):  # even heads -> parts 0:64, odd -> 64:128
                nc.sync.dma_start(
                    R_sb[64 * par: 64 * par + 64, :]
                    .rearrange("p (c n) -> p c n", n=N)
                    [:, c0:c1, nh * 512:nh * 512 + 512],
                    r_pairs[:, par * 512:par * 512 + 512][None, :, :]
                    .broadcast_to([64, nch, 512]),
                )

        def norm_stageC(nh, c0, c1):
            # per-chunk (512-wide) so no single DVE op piles onto the
            # queue ahead of the attention multiplies
            yh = (y_sb[:].rearrange("p (c n) -> p c n", n=N)
                  [:, c0:c1, nh * 512:(nh + 1) * 512])
            Rh = (R_sb[:].rearrange("p (c n) -> p c n", n=N)
                  [:, c0:c1, nh * 512:(nh + 1) * 512])
            nc.vector.tensor_mul(yh, yh, Rh)

        # staged-normalize / tail-prep slots: {(nh, pos, t): callable}
        norm_slots = {}
        post_slots = {}
        tail_psz = {}
        st = {}

        # nh0 chunks in order 0..5; nh1 in order 1..5,0 so the straggler
        # chunk is 0 and chunks [1,6) batch-normalize contiguously.
        for nh in range(2):
            order = list(range(NCC)) if nh == 0 else [1, 2, 3, 4, 5, 0]
            for pos, hp in enumerate(order):
                fn = chunk_dmas.pop((nh, pos), None)
                if fn is not None:
                    fn()
                hA, hB = 2 * hp, 2 * hp + 1
                u = pu_pool.tile([65, N], F32, tag="u", name=f"u_{nh}_{hp}")
                rhs_q = q_sb[:, hp * N + nh * 512: hp * N + nh * 512 + 512]
                for t in range(4):
                    for fn in norm_slots.pop((nh, pos, t), []):
                        fn()
                    ebAB = ebpool.tile([P, 2048], BF16, tag="eb")
                    nc.sync.dma_start(
                        ebAB[:], expBT[hp, :, nh * 8192 + t * 2048:
                                       nh * 8192 + (t + 1) * 2048])
                    psA = ps_pool.tile([P, N], F32, tag="ps", bufs=2,
                                       name=f"psA_{nh}_{hp}_{t}")
                    psB = ps_pool.tile([P, N], F32, tag="ps", bufs=2,
                                       name=f"psB_{nh}_{hp}_{t}")
                    for s in range(2):
                        j = 2 * t + s
                        kA = kpad_sb[:, hp * N + j * P: hp * N + (j + 1) * P]
                        kB = kpad_sb[:, (NCC + hp) * N + j * P: (NCC + hp) * N + (j + 1) * P]
                        nc.tensor.matmul(psA[:, s * 512:(s + 1) * 512], kA, rhs_q,
                                         start=True, stop=True)
                        nc.tensor.matmul(psB[:, s * 512:(s + 1) * 512], kB, rhs_q,
                                         start=True, stop=True)
                    # drip deferred matmul groups into the stream
                    if fill:
                        drip(fill.popleft())
                    for fn in post_slots.pop((nh, pos, t), []):
                        fn()
                    esAB = espool.tile([P, 2048], BF16, tag="esAB")
                    nc.scalar.activation(esAB[:, 0:1024], psA[:], EXPF)
                    nc.scalar.activation(esAB[:, 1024:2048], psB[:], EXPF)
                    pAB = ppool.tile([P, 2048], BF16, tag="pAB")
                    nc.vector.tensor_mul(pAB[:], esAB[:], ebAB[:])
                    for s in range(2):
                        j = 2 * t + s
                        vA = v_sb[:, (j * H + hA) * VS: (j * H + hA) * VS + 65]
                        vB = v_sb[:, (j * H + hB) * VS: (j * H + hB) * VS + 65]
                        nc.tensor.matmul(u[:, 0:512], vA, pAB[:, s * 512:(s + 1) * 512],
                                         start=(j == 0), stop=(j == NM - 1),
                                         skip_group_check=True)
                        nc.tensor.matmul(u[:, 512:1024], vB,
                                         pAB[:, 1024 + s * 512: 1024 + (s + 1) * 512],
                                         start=(j == 0), stop=(j == NM - 1),
                                         skip_group_check=True)
                # evacuate u via one copy; DMAs scatter y halves + sums
                # (u rows 0:64 = head dims, row 64 = sums)
                ys = hp * N + nh * 512
                stg = stgpool.tile([65, N], BF16, tag="stg")
                nc.vector.tensor_copy(stg[:], u[:])
                nc.sync.dma_start(y_sb[0:64, ys: ys + 512], stg[0:64, 0:512])
                nc.sync.dma_start(y_sb[64:128, ys: ys + 512], stg[0:64, 512:1024])
                nc.sync.dma_start(
                    r_acc[nh][16 * hp:16 * hp + 16, :],
                    stg[64:65, :])
                if nh == 0 and pos == NCC - 2:
                    # nh0 chunks 0-4 normalize staged into pos5's t-slots
                    norm_slots[(0, NCC - 1, 1)] = [
                        lambda: st.update(rd0=norm_stageA(0, 0, NCC - 1))]
                    norm_slots[(0, NCC - 1, 3)] = [
                        lambda: norm_stageB(0, st["rd0"], 0, NCC - 1)]
            if nh == 0:
                # nh0 normalize: batched recip/bounce done in pos5's slots;
                # the per-chunk y-multiplies spread one per slot across
                # nh1's first iterations (needed by slot 30's proj drips).
                # nh0 chunk 5's own bounce rides nh1-pos0's slots.
                norm_slots[(1, 0, 0)] = [lambda: norm_stageC(0, 0, 1)]
                norm_slots[(1, 0, 1)] = [
                    lambda: st.update(rd05=norm_stageA(0, NCC - 1, NCC)),
                    lambda: norm_stageC(0, 1, 2)]
                norm_slots[(1, 0, 2)] = [lambda: norm_stageC(0, 2, 3)]
                norm_slots[(1, 0, 3)] = [
                    lambda: norm_stageB(0, st["rd05"], NCC - 1, NCC),
                    lambda: norm_stageC(0, 3, 4)]
                norm_slots[(1, 1, 0)] = [lambda: norm_stageC(0, 4, 5)]
                norm_slots[(1, 1, 1)] = [lambda: norm_stageC(0, NCC - 1, NCC)]
                # nh1 normalize pipelined in two waves (chunks 1-4 during
                # pos4, chunk 5 during pos5) so the proj(1,*) cc1-5
                # partials can pre-run before straggler chunk 0 finishes
                norm_slots[(1, 4, 0)] = [
                    lambda: st.update(rd14=norm_stageA(1, 1, 5))]
                norm_slots[(1, 4, 1)] = [
                    lambda: norm_stageB(1, st["rd14"], 1, 5)]
                norm_slots[(1, 4, 2)] = [lambda: norm_stageC(1, 1, 2)]
                norm_slots[(1, 4, 3)] = [lambda: norm_stageC(1, 2, 3)]
                norm_slots[(1, 5, 0)] = [
                    lambda: norm_stageC(1, 3, 4),
                    lambda: st.update(rd15=norm_stageA(1, 5, 6))]
                norm_slots[(1, 5, 1)] = [
                    lambda: norm_stageC(1, 4, 5),
                    lambda: norm_stageB(1, st["rd15"], 5, 6)]
                norm_slots[(1, 5, 2)] = [lambda: norm_stageC(1, 5, 6)]
                norm_slots[(1, 5, 3)] = [
                    lambda: tail_psz.update(
                        g0=proj_partial(1, 0, tag="psz", last_cc=0))]
                post_slots[(1, 5, 3)] = [
                    lambda: tail_psz.update(
                        g1=proj_partial(1, 1, tag="psz", last_cc=0)),
                    lambda: tail_psz.update(
                        g2=proj_partial(1, 2, tag="ps", last_cc=0))]
                fill.append([])   # s29; proj halves land at s30-41
                for oc in range(NCC):
                    fill.append([("projA", oc)])
                    fill.append([("projB", oc)])
            else:
                # ---------------- tail ------------------------------------
                # Open the remaining proj partial (cc 1-5, independent of
                # the straggler chunk 0) so the PE chews on it while the
                # recip chain for chunk 0 runs on DVE/DMA.
                tail_psz["g3"] = proj_partial(1, 3, tag="ps", last_cc=0)
                # chunk 0 (heads 0,1) fast-path normalize: recip on the
                # [16,64] r_acc rows, broadcast via selector matmul into
                # PSUM (u's banks are free after the evac), y *= psR.
                r12_5 = rpool.tile([16, 64], BF16, tag="r125", name="r125")
                with nc.allow_low_precision(reason="bf16 recip of sums"):
                    nc.vector.reciprocal(r12_5[:], r_acc[1][0:16, :])
                nc.sync.dma_start(r12pad[0:1, :], r12_5[:])
                psR = pu_pool.tile([P, N], F32, tag="u", name="psR5")
                nc.tensor.matmul(psR[:, 0:512], sel_sb[:], r12pad[:, 0:512],
                                 start=True, stop=True)
                nc.tensor.matmul(psR[:, 512:1024], sel_sb[:],
                                 r12pad[:, 512:1024], start=True, stop=True)
                ys = 0 * N + nh * 512
                nc.vector.tensor_mul(y_sb[0:64, ys: ys + 512],
                                     y_sb[0:64, ys: ys + 512],
                                     psR[0:64, 0:512])
                nc.vector.tensor_mul(y_sb[64:128, ys: ys + 512],
                                     y_sb[64:128, ys: ys + 512],
                                     psR[64:128, 512:1024])
                # close the pre-opened groups; the last two run whole on
                # slots freed by psR (pu) and g0's evacuation (psz)
                for oc in range(4):
                    proj_final(tail_psz[f"g{oc}"], 1, oc, last_cc=0)
                g4 = proj_partial(1, 4, tag="u", pool=pu_pool, last_cc=0,
                                  bufs=1)
                proj_final(g4, 1, 4, last_cc=0)
                g5 = proj_partial(1, 5, tag="psz", last_cc=0)
                proj_final(g5, 1, 5, last_cc=0)
    return nc


_NC_CACHE = None


def _get_nc():
    global _NC_CACHE
    if _NC_CACHE is None:
        _NC_CACHE = build_nc()
        _NC_CACHE.finalize()   # Bacc: runs wait legalization + reg alloc
    return _NC_CACHE


def prep_inputs(x, rel_pos_bias, qkv_w, proj_w, proj_b):
    """Host-side (free) layout transforms -> per-core in_maps."""
    import ml_dtypes

    B = x.shape[0]
    W = np.array(qkv_w, dtype=np.float32)
    W[:C] *= SCALE  # fold q scaling into weights
    qkwT_h = (W[:2 * C].T.reshape(NCC, P, NOC_QK, P)
              .transpose(1, 2, 0, 3).reshape(P, NOC_QK * NCC * P)
              .astype(ml_dtypes.bfloat16))
    wvT_h = (W[2 * C:].T.reshape(NCC, P, C)
             .transpose(1, 0, 2).reshape(P, NCC * C).astype(ml_dtypes.bfloat16))
    pwT_h = (np.asarray(proj_w, np.float32).T.reshape(NCC, P, NCC, P)
             .transpose(1, 2, 0, 3).reshape(P, NCC * NCC * P)
             .astype(ml_dtypes.bfloat16))
    pb_h = np.asarray(proj_b, np.float32).reshape(NCC, P).T.copy()
    # exp(bias) interleaved per (hp, t): [hp, mp, nh, t, hs, s, c]
    # (hs = head A/B within the pair, j = 2t+s is the m-chunk)
    eb = np.exp(np.asarray(rel_pos_bias, np.float32)[0])          # [H, n, m]
    ebT = eb.transpose(0, 2, 1)                                   # [H, m, n]
    eb5 = ebT.reshape(NCC, 2, 4, 2, P, 2, 512)  # hp, hs, t, s, mp, nh, c
    expBT_h = np.ascontiguousarray(
        eb5.transpose(0, 4, 5, 2, 1, 3, 6)      # hp, mp, nh, t, hs, s, c
        .reshape(NCC, P, 2 * 4 * 2048)).astype(ml_dtypes.bfloat16)
    in_maps = []
    for b in range(B):
        xT_h = np.ascontiguousarray(
            np.asarray(x[b], np.float32).T.reshape(NCC, P, N)
            .transpose(1, 0, 2).reshape(P, NCC * N).astype(ml_dtypes.bfloat16))
        in_maps.append({
            "xT": xT_h, "qkwT": qkwT_h, "wvT": wvT_h,
            "expBT": expBT_h, "pwT": pwT_h, "pb": pb_h,
        })
    return in_maps


def postprocess(zT_arr):
    """[128, 6*1024] zT -> [1024, 768] output for one batch."""
    return (np.asarray(zT_arr, np.float32).reshape(P, NCC, N)
            .transpose(1, 0, 2).reshape(C, N).T.copy())


def kernel(x, rel_pos_bias, qkv_w, proj_w, proj_b):
    from concourse.bass_utils import run_bass_kernel_spmd

    nc = _get_nc()
    in_maps = prep_inputs(x, rel_pos_bias, qkv_w, proj_w, proj_b)
    res = run_bass_kernel_spmd(nc, in_maps, list(range(len(in_maps))))
    out = np.stack([postprocess(r["zT"]) for r in res.results])
    return out.astype(np.float32)
